# revision 12
# baseline (speedup 1.0000x reference)
"""MoE head (router + top-2 gated 8-expert ResidualMLP) on 8 Trainium2 cores.

Strategy: tokens are sharded across the 8 cores (4096 each); the router and
all weights are replicated. Top-2 sparsity is exploited by packing each
core's tokens per expert on the host (control-plane only — all tensor math
runs on device): for each core we build zT_packed[D, E*C] whose column block
e holds the (z-transposed) tokens routed to expert e, padded to a fixed
capacity C. The device computes, per core:
  - fp32 router: logits -> softmax(probs) -> top-2 (values+indices) -> gates
  - per-expert MLP on the packed tokens in fp32r (1 cyc/row on the PE):
      h1 = gelu(z @ W1[e] + b1[e]); x = h1 @ W2[e] + b2[e] + z
    and the layernorm+head collapsed to three per-token reductions
    (sum(x), sum(x^2), u = x . (gamma*Wo)) done as M<=3 matmuls, so
      ye = (u - mu * sum(gamma*Wo)) * rsqrt(var + eps) + (bo + beta . Wo)
  - partial sums over tokens for importance/load (ones-vector matmul)
The host combines: y[t] = w2[t,0]*ye[slot(t, top1)] + w2[t,1]*ye[slot(t, top2)],
concatenates the token-sharded outputs, and reduces the importance/load
partials. Dropping is impossible by construction: C is chosen at runtime
from the host routing replica with >=128 slack.
"""

import sys

import numpy as np

sys.path.insert(0, "/opt/trn_rl_repo")

import concourse.bass as bass  # noqa: E402
import concourse.mybir as mybir  # noqa: E402
from concourse import bacc  # noqa: E402
from concourse.tile import TileContext  # noqa: E402
from concourse.masks import make_identity  # noqa: E402
from concourse.bass_utils import run_bass_kernel_spmd  # noqa: E402

N, D, H, E, TOPK, OUT = 32768, 1024, 1024, 8, 2, 1
TAU = 1.2
LN_EPS = 1e-5
NCORES = 8
NTOK = N // NCORES           # tokens per core
P = 128                      # partitions
KC = D // P                  # contraction chunks (8)
HT = H // P                  # output row tiles (8)
TT = NTOK // P               # router token tiles per core (32)
RG = 4                       # router token tiles batched per group
F32 = mybir.dt.float32
F32R = mybir.dt.float32r
U32 = mybir.dt.uint32

_cache = {}


def _chunks(c):
    out = []
    s = 0
    while s < c:
        t = min(512, c - s)
        out.append((s, t))
        s += t
    return out


def build_kernel(C):
    nc = bacc.Bacc("TRN2", target_bir_lowering=False, debug=False)
    S = E * C

    # inputs (per core)
    zT_d = nc.dram_tensor("zT", [D, NTOK], F32, kind="ExternalInput")
    zTp_d = nc.dram_tensor("zTp", [D, S], F32R, kind="ExternalInput")
    w1_d = nc.dram_tensor("w1", [E, D, H], F32R, kind="ExternalInput")
    w2_d = nc.dram_tensor("w2", [E, H, H], F32R, kind="ExternalInput")
    wr_d = nc.dram_tensor("wr", [D, E], F32, kind="ExternalInput")
    br_d = nc.dram_tensor("br", [E], F32, kind="ExternalInput")
    b1_d = nc.dram_tensor("b1", [E, H], F32, kind="ExternalInput")
    b2_d = nc.dram_tensor("b2", [E, H], F32, kind="ExternalInput")
    # statlhs[e,:,0] = 1, statlhs[e,:,1] = (gamma[e]*Wo[e,:,0])
    sl_d = nc.dram_tensor("statlhs", [E, D, 2], F32R, kind="ExternalInput")
    # per-expert scalars: swo[e] = sum(gamma[e]*Wo[e]), bop[e] = bo[e]+beta[e].Wo[e]
    swo_d = nc.dram_tensor("swo", [E], F32, kind="ExternalInput")
    bop_d = nc.dram_tensor("bop", [E], F32, kind="ExternalInput")

    # outputs (per core)
    probs_o = nc.dram_tensor("probs", [NTOK, E], F32, kind="ExternalOutput")
    idx_o = nc.dram_tensor("topk", [NTOK, 2], mybir.dt.int32, kind="ExternalOutput")
    w2g_o = nc.dram_tensor("gates", [NTOK, 2], F32, kind="ExternalOutput")
    ye_o = nc.dram_tensor("ye", [E, C], F32, kind="ExternalOutput")
    il_o = nc.dram_tensor("implood", [1, TT * 2 * E], F32, kind="ExternalOutput")

    with TileContext(nc) as tc:
        with tc.tile_pool(name="consts", bufs=1) as consts, \
             tc.tile_pool(name="wpool", bufs=1) as wpool, \
             tc.tile_pool(name="act", bufs=2) as act, \
             tc.tile_pool(name="small", bufs=3) as small, \
             tc.tile_pool(name="rt", bufs=2) as rt, \
             tc.tile_pool(name="stats", bufs=1) as stats, \
             tc.tile_pool(name="ps_h1", bufs=2, space="PSUM") as ps_h1, \
             tc.tile_pool(name="ps_h2", bufs=2, space="PSUM") as ps_h2, \
             tc.tile_pool(name="ps_st", bufs=1, space="PSUM") as ps_st, \
             tc.tile_pool(name="ps_sq", bufs=1, space="PSUM") as ps_sq, \
             tc.tile_pool(name="ps_rt", bufs=1, space="PSUM") as ps_rt, \
             tc.tile_pool(name="ps_misc", bufs=1, space="PSUM") as ps_misc:

            # ---------- constants ----------
            wr_t = consts.tile([P, KC, E], F32)
            nc.sync.dma_start(out=wr_t, in_=wr_d.rearrange("(k p) e -> p k e", p=P))
            br_b = consts.tile([P, E], F32)
            nc.sync.dma_start(out=br_b, in_=bass.AP(tensor=br_d, offset=0, ap=[[0, P], [1, E]]))
            swo_t = consts.tile([E, 1], F32)
            nc.sync.dma_start(out=swo_t, in_=sl_wrap(swo_d))
            bop_t = consts.tile([E, 1], F32)
            nc.sync.dma_start(out=bop_t, in_=sl_wrap(bop_d))

            # stats accumulators [E, C] (partition = expert)
            sumx_all = stats.tile([E, C], F32)
            u_all = stats.tile([E, C], F32)
            sumsq_all = stats.tile([E, C], F32)

            ones1 = consts.tile([P, 1], F32)
            nc.vector.memset(ones1, 1.0)
            ident8 = consts.tile([8, 8], F32)
            make_identity(nc, ident8)

            # ---------- router (expert-major fp32 matmul, DMA-transposed) ----------
            zr = zT_d.rearrange("(k p) (g t) -> p k g t", p=P, t=P)  # [P,KC,TT,P]
            n_mm = 0
            for g0 in range(0, TT, RG):
                gn = min(RG, TT - g0)
                gP = gn * P
                ztile = rt.tile([P, KC, RG * P], F32, tag="zrt")
                nc.sync.dma_start(
                    out=ztile[:, :, :gP],
                    in_=zr[:, :, g0 : g0 + gn, :].rearrange("p k g t -> p k (g t)"),
                )
                lg_ps = ps_rt.tile([8, RG * P], F32, tag="lgps")
                for k in range(KC):
                    nc.tensor.matmul(
                        lg_ps[:, :gP], wr_t[:, k, :], ztile[:, k, :gP],
                        start=(k == 0), stop=(k == KC - 1),
                    )
                lg_em = rt.tile([8, RG * P], F32, tag="lgem")
                nc.vector.tensor_copy(lg_em[:, :gP], lg_ps[:, :gP])
                lg_sb = rt.tile([P, RG, E], F32, tag="lg")
                for gi in range(gn):
                    tp_ps = ps_misc.tile([P, 8], F32, tag="rtmp")
                    nc.tensor.transpose(tp_ps, lg_em[:, gi * P : (gi + 1) * P], ident8)
                    nc.vector.tensor_add(lg_sb[:, gi, :], tp_ps, br_b)
                # softmax over E for the whole group
                mx = rt.tile([P, RG, 1], F32, tag="mx")
                nc.vector.reduce_max(mx[:, :gn, :], lg_sb[:, :gn, :], axis=mybir.AxisListType.X)
                bm = rt.tile([P, RG, 1], F32, tag="bm")
                nc.vector.tensor_scalar_mul(bm[:, :gn, :], mx[:, :gn, :], -1.0 / TAU)
                ex = rt.tile([P, RG, E], F32, tag="ex")
                for gi in range(gn):
                    nc.scalar.activation(
                        ex[:, gi, :], lg_sb[:, gi, :],
                        mybir.ActivationFunctionType.Exp,
                        bias=bm[:, gi, :], scale=1.0 / TAU,
                    )
                sm = rt.tile([P, RG, 1], F32, tag="sm")
                nc.vector.reduce_sum(sm[:, :gn, :], ex[:, :gn, :], axis=mybir.AxisListType.X)
                rs = rt.tile([P, RG, 1], F32, tag="rs")
                nc.vector.reciprocal(rs[:, :gn, :], sm[:, :gn, :])
                probs = rt.tile([P, RG, E], F32, tag="probs")
                mask = rt.tile([P, RG, E], F32, tag="mask")
                vals = rt.tile([P, RG, 8], F32, tag="vals")
                idxs = rt.tile([P, RG, 8], U32, tag="idxs")
                den = rt.tile([P, RG, 1], F32, tag="den")
                w2g = rt.tile([P, RG, 2], F32, tag="w2g")
                pm_all = rt.tile([P, RG, 2 * E], F32, tag="pm")
                for gi in range(gn):
                    nc.vector.tensor_scalar(
                        probs[:, gi, :], ex[:, gi, :], rs[:, gi, :],
                        scalar2=None, op0=mybir.AluOpType.mult,
                    )
                    nc.vector.max_with_indices(vals[:, gi, :], idxs[:, gi, :], probs[:, gi, :])
                    nc.vector.tensor_add(den[:, gi, :], vals[:, gi, 0:1], vals[:, gi, 1:2])
                    nc.vector.tensor_scalar_max(den[:, gi, :], den[:, gi, :], 1e-8)
                    nc.vector.reciprocal(den[:, gi, :], den[:, gi, :])
                    nc.vector.tensor_scalar(
                        w2g[:, gi, :], vals[:, gi, 0:2], den[:, gi, :],
                        scalar2=None, op0=mybir.AluOpType.mult,
                    )
                    nc.vector.tensor_scalar(
                        mask[:, gi, :], probs[:, gi, :], vals[:, gi, 1:2],
                        scalar2=None, op0=mybir.AluOpType.is_ge,
                    )
                    nc.vector.tensor_copy(pm_all[:, gi, :E], probs[:, gi, :])
                    nc.vector.tensor_copy(pm_all[:, gi, E:], mask[:, gi, :])
                ilg_ps = ps_misc.tile([1, RG * 2 * E], F32, tag="rtmp")
                nc.tensor.matmul(ilg_ps[:, : gn * 2 * E], ones1, pm_all[:, :gn, :],
                                 start=True, stop=True)
                il_stage = rt.tile([1, RG * 2 * E], F32, tag="ilstg")
                nc.vector.tensor_copy(il_stage[:, : gn * 2 * E], ilg_ps[:, : gn * 2 * E])
                nc.sync.dma_start(
                    out=il_o[0:1, g0 * 2 * E : (g0 + gn) * 2 * E],
                    in_=il_stage[:, : gn * 2 * E],
                )
                pr_v = probs_o.rearrange("(g t) e -> t g e", t=P)
                nc.sync.dma_start(out=pr_v[:, g0 : g0 + gn, :], in_=probs[:, :gn, :])
                ix_v = idx_o.rearrange("(g t) e -> t g e", t=P)
                nc.gpsimd.dma_start(out=ix_v[:, g0 : g0 + gn, :], in_=idxs[:, :gn, 0:2])
                wg_v = w2g_o.rearrange("(g t) e -> t g e", t=P)
                nc.sync.dma_start(out=wg_v[:, g0 : g0 + gn, :], in_=w2g[:, :gn, :])

            # ---------- experts on packed tokens (fp32r) ----------
            zp_r = zTp_d.rearrange("(k p) s -> p k s", p=P)
            w1_r = w1_d.rearrange("e (k p) h -> e p k h", p=P)
            w2_r = w2_d.rearrange("e (k p) h -> e p k h", p=P)
            b1_r = b1_d.rearrange("e (h p) -> e p h", p=P)
            b2_r = b2_d.rearrange("e (h p) -> e p h", p=P)
            sl_r = sl_d.rearrange("e (k p) c -> e p k c", p=P)

            for e in range(E):
                w1t = wpool.tile([P, KC, H], F32R, tag="w1")
                nc.sync.dma_start(out=w1t, in_=w1_r[e])
                w2t = wpool.tile([P, KC, H], F32R, tag="w2")
                nc.sync.dma_start(out=w2t, in_=w2_r[e])
                b1t = small.tile([P, HT], F32, tag="b1")
                nc.sync.dma_start(out=b1t, in_=b1_r[e])
                b2t = small.tile([P, HT], F32, tag="b2")
                nc.sync.dma_start(out=b2t, in_=b2_r[e])
                slt = small.tile([P, KC, 2], F32R, tag="sl")
                nc.sync.dma_start(out=slt, in_=sl_r[e])

                for (c0, tw) in _chunks(C):
                    zp = act.tile([P, KC, 512], F32R, tag="zp")
                    nc.sync.dma_start(
                        out=zp[:, :, :tw], in_=zp_r[:, :, e * C + c0 : e * C + c0 + tw]
                    )
                    h1g = act.tile([P, KC, 512], F32R, tag="h1g")
                    for ht in range(HT):
                        h1_ps = ps_h1.tile([P, 512], F32, tag="h1ps")
                        for k in range(KC):
                            nc.tensor.matmul(
                                h1_ps[:, :tw],
                                w1t[:, k, ht * P : (ht + 1) * P],
                                zp[:, k, :tw],
                                start=(k == 0),
                                stop=(k == KC - 1),
                            )
                        nc.scalar.activation(
                            h1g[:, ht, :tw], h1_ps[:, :tw],
                            mybir.ActivationFunctionType.Gelu,
                            bias=b1t[:, ht : ht + 1], scale=1.0,
                        )
                    st_ps = ps_st.tile([2, 512], F32, tag="stps")
                    sq_ps = ps_sq.tile([1, 512], F32, tag="sqps")
                    for ht in range(HT):
                        h2_ps = ps_h2.tile([P, 512], F32, tag="h2ps")
                        for k in range(KC):
                            nc.tensor.matmul(
                                h2_ps[:, :tw],
                                w2t[:, k, ht * P : (ht + 1) * P],
                                h1g[:, k, :tw],
                                start=(k == 0),
                                stop=(k == KC - 1),
                            )
                        # x = h2 + b2 + z  (in place into zp[ht])
                        tmp = act.tile([P, 512], F32, tag="tmp")
                        nc.scalar.activation(
                            tmp[:, :tw], h2_ps[:, :tw],
                            mybir.ActivationFunctionType.Identity,
                            bias=b2t[:, ht : ht + 1], scale=1.0,
                        )
                        nc.vector.tensor_add(zp[:, ht, :tw], zp[:, ht, :tw], tmp[:, :tw])
                    # stats: [sumx; u] then sumsq
                    for k in range(KC):
                        nc.tensor.matmul(
                            st_ps[0:2, :tw], slt[:, k, :], zp[:, k, :tw],
                            start=(k == 0), stop=(k == KC - 1),
                        )
                    for k in range(KC):
                        x2t = act.tile([P, 512], F32R, tag="x2")
                        nc.vector.tensor_mul(x2t[:, :tw], zp[:, k, :tw], zp[:, k, :tw])
                        nc.tensor.matmul(
                            sq_ps[0:1, :tw], slt[:, k, 0:1], x2t[:, :tw],
                            start=(k == 0), stop=(k == KC - 1),
                        )
                    stg_su = act.tile([2, 512], F32, tag="stgsu")
                    stg_sq = act.tile([1, 512], F32, tag="stgsq")
                    nc.vector.tensor_copy(stg_su[:, :tw], st_ps[:, :tw])
                    nc.vector.tensor_copy(stg_sq[:, :tw], sq_ps[:, :tw])
                    nc.sync.dma_start(out=sumx_all[e : e + 1, c0 : c0 + tw], in_=stg_su[0:1, :tw])
                    nc.sync.dma_start(out=u_all[e : e + 1, c0 : c0 + tw], in_=stg_su[1:2, :tw])
                    nc.sync.dma_start(out=sumsq_all[e : e + 1, c0 : c0 + tw], in_=stg_sq[:, :tw])

            # ---------- final per-slot LN + head math on [E, C] ----------
            # in-place: mu lives in sumx_all, var in sumsq_all, rstd in tmp_s
            tmp_s = stats.tile([E, C], F32)
            eps_t = stats.tile([E, 1], F32)
            nc.vector.memset(eps_t, LN_EPS)
            nc.vector.tensor_scalar_mul(sumx_all, sumx_all, 1.0 / H)   # mu
            nc.vector.tensor_scalar_mul(sumsq_all, sumsq_all, 1.0 / H) # E[x^2]
            nc.vector.tensor_mul(tmp_s, sumx_all, sumx_all)            # mu^2
            nc.vector.tensor_sub(sumsq_all, sumsq_all, tmp_s)          # var
            nc.scalar.activation(tmp_s, sumsq_all, mybir.ActivationFunctionType.Sqrt,
                                 bias=eps_t, scale=1.0)
            nc.vector.reciprocal(tmp_s, tmp_s)                         # rstd
            # ye = (u - mu*swo) * rstd + bop
            nc.vector.tensor_scalar(sumx_all, sumx_all, swo_t, scalar2=None,
                                    op0=mybir.AluOpType.mult)
            nc.vector.tensor_sub(u_all, u_all, sumx_all)
            nc.vector.tensor_mul(u_all, u_all, tmp_s)
            nc.vector.tensor_scalar(u_all, u_all, bop_t, scalar2=None,
                                    op0=mybir.AluOpType.add)
            nc.sync.dma_start(out=ye_o[:, :], in_=u_all)

    nc.compile()
    return nc


def sl_wrap(d):
    return d.rearrange("(e one) -> e one", one=1)


def kernel(z, Wr, br, W1, b1, W2, b2, gamma, beta, Wo, bo):
    z = np.ascontiguousarray(np.asarray(z, dtype=np.float32))
    Wr = np.asarray(Wr, dtype=np.float32)
    br = np.asarray(br, dtype=np.float32)
    W1 = np.asarray(W1, dtype=np.float32)
    b1 = np.asarray(b1, dtype=np.float32)
    W2 = np.asarray(W2, dtype=np.float32)
    b2 = np.asarray(b2, dtype=np.float32)
    gamma = np.asarray(gamma, dtype=np.float32)
    beta = np.asarray(beta, dtype=np.float32)
    Wo = np.asarray(Wo, dtype=np.float32)
    bo = np.asarray(bo, dtype=np.float32)

    # ---- host routing replica (control plane: packing only) ----
    logits = z @ Wr + br
    m = logits.max(-1, keepdims=True)
    ex = np.exp((logits - m) / TAU)
    probs_h = ex / ex.sum(-1, keepdims=True)
    order = np.argsort(-probs_h, axis=-1, kind="stable")
    top2 = order[:, :2]

    counts = np.zeros((NCORES, E), dtype=np.int64)
    for c in range(NCORES):
        t2 = top2[c * NTOK : (c + 1) * NTOK]
        for e in range(E):
            counts[c, e] = (t2 == e).sum()
    C = int(((counts.max() + 128 + 127) // 128) * 128)

    # packed layouts + slot maps
    zT = np.ascontiguousarray(z.T)  # [D, N]
    gwo = (gamma * Wo[:, :, 0])  # [E, H]
    statlhs = np.stack([np.ones((E, D), np.float32), gwo], axis=-1).astype(np.float32)
    swo = gwo.sum(-1).astype(np.float32)
    bop = (bo[:, 0] + (beta * Wo[:, :, 0]).sum(-1)).astype(np.float32)

    in_maps = []
    slot_of = np.zeros((N, 2), dtype=np.int64)  # global slot per (token, k)
    for c in range(NCORES):
        lo = c * NTOK
        t2 = top2[lo : lo + NTOK]
        zTc = zT[:, lo : lo + NTOK]
        zTp = np.zeros((D, E * C), dtype=np.float32)
        for e in range(E):
            rows, ks = np.nonzero(t2 == e)
            zTp[:, e * C : e * C + rows.size] = zTc[:, rows]
            slot_of[lo + rows, ks] = c * (E * C) + e * C + np.arange(rows.size)
        in_maps.append({
            "zT": zTc.copy(), "zTp": zTp, "w1": W1, "w2": W2, "wr": Wr,
            "br": br, "b1": b1, "b2": b2, "statlhs": statlhs,
            "swo": swo, "bop": bop,
        })

    key = ("nc", C)
    if key not in _cache:
        _cache[key] = build_kernel(C)
    nc = _cache[key]
    global _last_in_maps
    _last_in_maps = in_maps

    res = run_bass_kernel_spmd(nc, in_maps, core_ids=list(range(NCORES)))

    probs = np.concatenate([r["probs"] for r in res.results], axis=0)
    topk_idx = np.concatenate([r["topk"] for r in res.results], axis=0).astype(np.int32)
    gates = np.concatenate([r["gates"] for r in res.results], axis=0)
    ye = np.concatenate([r["ye"].reshape(-1) for r in res.results], axis=0)  # [NCORES*E*C]
    il = np.stack([r["implood"][0].reshape(TT, 2 * E).sum(0) for r in res.results], axis=0)

    y_hat = (gates[:, 0] * ye[slot_of[:, 0]] + gates[:, 1] * ye[slot_of[:, 1]])
    y_hat = y_hat.astype(np.float32)[:, None]
    importance = (il[:, :E].sum(0) / N).astype(np.float32)
    load = (il[:, E:].sum(0) / N).astype(np.float32)
    return y_hat, probs, topk_idx, importance, load


# revision 14
# speedup vs baseline: 1.0701x; 1.0701x over previous
"""MoE head (router + top-2 gated 8-expert ResidualMLP) on 8 Trainium2 cores.

Strategy: tokens are sharded across the 8 cores (4096 each); the router and
all weights are replicated. Top-2 sparsity is exploited by packing each
core's tokens per expert on the host (control-plane only — all tensor math
runs on device): for each core we build zT_packed[D, E*C] whose column block
e holds the (z-transposed) tokens routed to expert e, padded to a fixed
capacity C. The device computes, per core:
  - fp32 router: logits -> softmax(probs) -> top-2 (values+indices) -> gates
  - per-expert MLP on the packed tokens in fp32r (1 cyc/row on the PE):
      h1 = gelu(z @ W1[e] + b1[e]); x = h1 @ W2[e] + b2[e] + z
    and the layernorm+head collapsed to three per-token reductions
    (sum(x), sum(x^2), u = x . (gamma*Wo)) done as M<=3 matmuls, so
      ye = (u - mu * sum(gamma*Wo)) * rsqrt(var + eps) + (bo + beta . Wo)
  - partial sums over tokens for importance/load (ones-vector matmul)
The host combines: y[t] = w2[t,0]*ye[slot(t, top1)] + w2[t,1]*ye[slot(t, top2)],
concatenates the token-sharded outputs, and reduces the importance/load
partials. Dropping is impossible by construction: C is chosen at runtime
from the host routing replica with >=128 slack.
"""

import sys

import numpy as np

sys.path.insert(0, "/opt/trn_rl_repo")

import concourse.bass as bass  # noqa: E402
import concourse.mybir as mybir  # noqa: E402
from concourse import bacc  # noqa: E402
from concourse.tile import TileContext  # noqa: E402
from concourse.bass_utils import run_bass_kernel_spmd  # noqa: E402

N, D, H, E, TOPK, OUT = 32768, 1024, 1024, 8, 2, 1
TAU = 1.2
LN_EPS = 1e-5
NCORES = 8
NTOK = N // NCORES           # tokens per core
P = 128                      # partitions
KC = D // P                  # contraction chunks (8)
HT = H // P                  # output row tiles (8)
TT = NTOK // P               # router token tiles per core (32)
RG = 4                       # router token tiles batched per group
F32 = mybir.dt.float32
F32R = mybir.dt.float32r
U32 = mybir.dt.uint32

_cache = {}


def _chunks(c):
    out = []
    s = 0
    while s < c:
        t = min(512, c - s)
        out.append((s, t))
        s += t
    return out


def build_kernel(C):
    nc = bacc.Bacc("TRN2", target_bir_lowering=False, debug=False)
    S = E * C

    # inputs (per core)
    NG = TT // RG
    zrt_d = nc.dram_tensor("zrt", [NG, P, KC, RG * P], F32, kind="ExternalInput")
    zTp_d = nc.dram_tensor("zTp", [D * S], F32R, kind="ExternalInput")
    w1_d = nc.dram_tensor("w1", [E, P, KC, H], F32R, kind="ExternalInput")
    w2_d = nc.dram_tensor("w2", [E, P, KC, H], F32R, kind="ExternalInput")
    wr_d = nc.dram_tensor("wr", [D, E], F32, kind="ExternalInput")
    br_d = nc.dram_tensor("br", [E], F32, kind="ExternalInput")
    b1_d = nc.dram_tensor("b1", [E, P, HT], F32, kind="ExternalInput")
    b2_d = nc.dram_tensor("b2", [E, P, HT], F32, kind="ExternalInput")
    # statlhs[e,:,0] = 1, statlhs[e,:,1] = (gamma[e]*Wo[e,:,0])
    sl_d = nc.dram_tensor("statlhs", [E, P, KC, 2], F32R, kind="ExternalInput")
    # per-expert scalars: swo[e] = sum(gamma[e]*Wo[e]), bop[e] = bo[e]+beta[e].Wo[e]
    swo_d = nc.dram_tensor("swo", [E], F32, kind="ExternalInput")
    bop_d = nc.dram_tensor("bop", [E], F32, kind="ExternalInput")

    # outputs (per core)
    probs_o = nc.dram_tensor("probs", [NTOK, E], F32, kind="ExternalOutput")
    idx_o = nc.dram_tensor("topk", [NTOK, 2], mybir.dt.int32, kind="ExternalOutput")
    w2g_o = nc.dram_tensor("gates", [NTOK, 2], F32, kind="ExternalOutput")
    ye_o = nc.dram_tensor("ye", [E, C], F32, kind="ExternalOutput")
    il_o = nc.dram_tensor("implood", [1, TT * 2 * E], F32, kind="ExternalOutput")

    with TileContext(nc) as tc:
        with tc.tile_pool(name="consts", bufs=1) as consts, \
             tc.tile_pool(name="wpool", bufs=1) as wpool, \
             tc.tile_pool(name="act", bufs=2) as act, \
             tc.tile_pool(name="small", bufs=3) as small, \
             tc.tile_pool(name="rt", bufs=2) as rt, \
             tc.tile_pool(name="stats", bufs=1) as stats, \
             tc.tile_pool(name="ps_h1", bufs=2, space="PSUM") as ps_h1, \
             tc.tile_pool(name="ps_h2", bufs=2, space="PSUM") as ps_h2, \
             tc.tile_pool(name="ps_st", bufs=1, space="PSUM") as ps_st, \
             tc.tile_pool(name="ps_sq", bufs=1, space="PSUM") as ps_sq, \
             tc.tile_pool(name="ps_rt", bufs=1, space="PSUM") as ps_rt, \
             tc.tile_pool(name="ps_misc", bufs=1, space="PSUM") as ps_misc:

            # ---------- constants ----------
            wr_t = consts.tile([P, KC, E], F32)
            nc.sync.dma_start(out=wr_t, in_=wr_d.rearrange("(k p) e -> p k e", p=P))
            br_b = consts.tile([P, E], F32)
            nc.sync.dma_start(out=br_b, in_=bass.AP(tensor=br_d, offset=0, ap=[[0, P], [1, E]]))
            swo_t = consts.tile([E, 1], F32)
            nc.sync.dma_start(out=swo_t, in_=sl_wrap(swo_d))
            bop_t = consts.tile([E, 1], F32)
            nc.sync.dma_start(out=bop_t, in_=sl_wrap(bop_d))

            # stats accumulators [E, C] (partition = expert)
            sumx_all = stats.tile([E, C], F32)
            u_all = stats.tile([E, C], F32)
            sumsq_all = stats.tile([E, C], F32)

            ones1 = consts.tile([P, 1], F32)
            nc.vector.memset(ones1, 1.0)

            # ---------- router (token-major fp32: bit-stable vs reference) ----------
            n_groups = TT // RG
            for g in range(n_groups):
                g0 = g * RG
                gn = RG
                gP = gn * P
                ztile = rt.tile([P, KC, RG * P], F32, tag="zrt")
                nc.sync.dma_start(out=ztile, in_=zrt_d[g])
                lg_sb = rt.tile([P, RG, E], F32, tag="lg")
                for gi in range(gn):
                    lg_ps = ps_rt.tile([P, E], F32, tag="lgps")
                    for k in range(KC):
                        nc.tensor.matmul(
                            lg_ps,
                            ztile[:, k, gi * P : (gi + 1) * P],
                            wr_t[:, k, :],
                            start=(k == 0),
                            stop=(k == KC - 1),
                        )
                    nc.vector.tensor_add(lg_sb[:, gi, :], lg_ps, br_b)
                # softmax over E for the whole group
                mx = rt.tile([P, RG, 1], F32, tag="mx")
                nc.vector.reduce_max(mx[:, :gn, :], lg_sb[:, :gn, :], axis=mybir.AxisListType.X)
                bm = rt.tile([P, RG, 1], F32, tag="bm")
                nc.vector.tensor_scalar_mul(bm[:, :gn, :], mx[:, :gn, :], -1.0 / TAU)
                ex = rt.tile([P, RG, E], F32, tag="ex")
                for gi in range(gn):
                    nc.scalar.activation(
                        ex[:, gi, :], lg_sb[:, gi, :],
                        mybir.ActivationFunctionType.Exp,
                        bias=bm[:, gi, :], scale=1.0 / TAU,
                    )
                sm = rt.tile([P, RG, 1], F32, tag="sm")
                nc.vector.reduce_sum(sm[:, :gn, :], ex[:, :gn, :], axis=mybir.AxisListType.X)
                rs = rt.tile([P, RG, 1], F32, tag="rs")
                nc.vector.reciprocal(rs[:, :gn, :], sm[:, :gn, :])
                probs = rt.tile([P, RG, E], F32, tag="probs")
                mask = rt.tile([P, RG, E], F32, tag="mask")
                vals = rt.tile([P, RG, 8], F32, tag="vals")
                idxs = rt.tile([P, RG, 8], U32, tag="idxs")
                den = rt.tile([P, RG, 1], F32, tag="den")
                w2g = rt.tile([P, RG, 2], F32, tag="w2g")
                pm_all = rt.tile([P, RG, 2 * E], F32, tag="pm")
                for gi in range(gn):
                    nc.vector.tensor_scalar(
                        probs[:, gi, :], ex[:, gi, :], rs[:, gi, :],
                        scalar2=None, op0=mybir.AluOpType.mult,
                    )
                    nc.vector.max_with_indices(vals[:, gi, :], idxs[:, gi, :], probs[:, gi, :])
                    nc.vector.tensor_add(den[:, gi, :], vals[:, gi, 0:1], vals[:, gi, 1:2])
                    nc.vector.tensor_scalar_max(den[:, gi, :], den[:, gi, :], 1e-8)
                    nc.vector.reciprocal(den[:, gi, :], den[:, gi, :])
                    nc.vector.tensor_scalar(
                        w2g[:, gi, :], vals[:, gi, 0:2], den[:, gi, :],
                        scalar2=None, op0=mybir.AluOpType.mult,
                    )
                    nc.vector.tensor_scalar(
                        mask[:, gi, :], probs[:, gi, :], vals[:, gi, 1:2],
                        scalar2=None, op0=mybir.AluOpType.is_ge,
                    )
                    nc.vector.tensor_copy(pm_all[:, gi, :E], probs[:, gi, :])
                    nc.vector.tensor_copy(pm_all[:, gi, E:], mask[:, gi, :])
                ilg_ps = ps_misc.tile([1, RG * 2 * E], F32, tag="rtmp")
                nc.tensor.matmul(ilg_ps[:, : gn * 2 * E], ones1, pm_all[:, :gn, :],
                                 start=True, stop=True)
                il_stage = rt.tile([1, RG * 2 * E], F32, tag="ilstg")
                nc.vector.tensor_copy(il_stage[:, : gn * 2 * E], ilg_ps[:, : gn * 2 * E])
                nc.sync.dma_start(
                    out=il_o[0:1, g0 * 2 * E : (g0 + gn) * 2 * E],
                    in_=il_stage[:, : gn * 2 * E],
                )
                pr_v = probs_o.rearrange("(g t) e -> t g e", t=P)
                nc.sync.dma_start(out=pr_v[:, g0 : g0 + gn, :], in_=probs[:, :gn, :])
                ix_v = idx_o.rearrange("(g t) e -> t g e", t=P)
                nc.gpsimd.dma_start(out=ix_v[:, g0 : g0 + gn, :], in_=idxs[:, :gn, 0:2])
                wg_v = w2g_o.rearrange("(g t) e -> t g e", t=P)
                nc.sync.dma_start(out=wg_v[:, g0 : g0 + gn, :], in_=w2g[:, :gn, :])

            # ---------- experts on packed tokens (fp32r) ----------

            for e in range(E):
                w1t = wpool.tile([P, KC, H], F32R, tag="w1")
                nc.sync.dma_start(out=w1t, in_=w1_d[e])
                w2t = wpool.tile([P, KC, H], F32R, tag="w2")
                nc.sync.dma_start(out=w2t, in_=w2_d[e])
                b1t = small.tile([P, HT], F32, tag="b1")
                nc.sync.dma_start(out=b1t, in_=b1_d[e])
                b2t = small.tile([P, HT], F32, tag="b2")
                nc.sync.dma_start(out=b2t, in_=b2_d[e])
                slt = small.tile([P, KC, 2], F32R, tag="sl")
                nc.sync.dma_start(out=slt, in_=sl_d[e])

                for (c0, tw) in _chunks(C):
                    zp = act.tile([P, KC, 512], F32R, tag="zp")
                    off = (e * C + c0) * D
                    nc.sync.dma_start(
                        out=zp[:, :, :tw],
                        in_=zTp_d[off : off + P * KC * tw].rearrange(
                            "(p k t) -> p k t", p=P, k=KC
                        ),
                    )
                    h1g = act.tile([P, KC, 512], F32R, tag="h1g")
                    for ht in range(HT):
                        h1_ps = ps_h1.tile([P, 512], F32, tag="h1ps")
                        for k in range(KC):
                            nc.tensor.matmul(
                                h1_ps[:, :tw],
                                w1t[:, k, ht * P : (ht + 1) * P],
                                zp[:, k, :tw],
                                start=(k == 0),
                                stop=(k == KC - 1),
                            )
                        nc.scalar.activation(
                            h1g[:, ht, :tw], h1_ps[:, :tw],
                            mybir.ActivationFunctionType.Gelu,
                            bias=b1t[:, ht : ht + 1], scale=1.0,
                        )
                    st_ps = ps_st.tile([2, 512], F32, tag="stps")
                    sq_ps = ps_sq.tile([1, 512], F32, tag="sqps")
                    for ht in range(HT):
                        h2_ps = ps_h2.tile([P, 512], F32, tag="h2ps")
                        for k in range(KC):
                            nc.tensor.matmul(
                                h2_ps[:, :tw],
                                w2t[:, k, ht * P : (ht + 1) * P],
                                h1g[:, k, :tw],
                                start=(k == 0),
                                stop=(k == KC - 1),
                            )
                        # x = h2 + b2 + z  (in place into zp[ht])
                        tmp = act.tile([P, 512], F32, tag="tmp")
                        nc.scalar.activation(
                            tmp[:, :tw], h2_ps[:, :tw],
                            mybir.ActivationFunctionType.Identity,
                            bias=b2t[:, ht : ht + 1], scale=1.0,
                        )
                        nc.vector.tensor_add(zp[:, ht, :tw], zp[:, ht, :tw], tmp[:, :tw])
                    # stats: [sumx; u] then sumsq
                    for k in range(KC):
                        nc.tensor.matmul(
                            st_ps[0:2, :tw], slt[:, k, :], zp[:, k, :tw],
                            start=(k == 0), stop=(k == KC - 1),
                        )
                    for k in range(KC):
                        x2t = act.tile([P, 512], F32R, tag="x2")
                        nc.vector.tensor_mul(x2t[:, :tw], zp[:, k, :tw], zp[:, k, :tw])
                        nc.tensor.matmul(
                            sq_ps[0:1, :tw], slt[:, k, 0:1], x2t[:, :tw],
                            start=(k == 0), stop=(k == KC - 1),
                        )
                    stg_su = act.tile([2, 512], F32, tag="stgsu")
                    stg_sq = act.tile([1, 512], F32, tag="stgsq")
                    nc.vector.tensor_copy(stg_su[:, :tw], st_ps[:, :tw])
                    nc.vector.tensor_copy(stg_sq[:, :tw], sq_ps[:, :tw])
                    nc.sync.dma_start(out=sumx_all[e : e + 1, c0 : c0 + tw], in_=stg_su[0:1, :tw])
                    nc.sync.dma_start(out=u_all[e : e + 1, c0 : c0 + tw], in_=stg_su[1:2, :tw])
                    nc.sync.dma_start(out=sumsq_all[e : e + 1, c0 : c0 + tw], in_=stg_sq[:, :tw])

            # ---------- final per-slot LN + head math on [E, C] ----------
            # in-place: mu lives in sumx_all, var in sumsq_all, rstd in tmp_s
            tmp_s = stats.tile([E, C], F32)
            eps_t = stats.tile([E, 1], F32)
            nc.vector.memset(eps_t, LN_EPS)
            nc.vector.tensor_scalar_mul(sumx_all, sumx_all, 1.0 / H)   # mu
            nc.vector.tensor_scalar_mul(sumsq_all, sumsq_all, 1.0 / H) # E[x^2]
            nc.vector.tensor_mul(tmp_s, sumx_all, sumx_all)            # mu^2
            nc.vector.tensor_sub(sumsq_all, sumsq_all, tmp_s)          # var
            nc.scalar.activation(tmp_s, sumsq_all, mybir.ActivationFunctionType.Sqrt,
                                 bias=eps_t, scale=1.0)
            nc.vector.reciprocal(tmp_s, tmp_s)                         # rstd
            # ye = (u - mu*swo) * rstd + bop
            nc.vector.tensor_scalar(sumx_all, sumx_all, swo_t, scalar2=None,
                                    op0=mybir.AluOpType.mult)
            nc.vector.tensor_sub(u_all, u_all, sumx_all)
            nc.vector.tensor_mul(u_all, u_all, tmp_s)
            nc.vector.tensor_scalar(u_all, u_all, bop_t, scalar2=None,
                                    op0=mybir.AluOpType.add)
            nc.sync.dma_start(out=ye_o[:, :], in_=u_all)

    nc.compile()
    return nc


def sl_wrap(d):
    return d.rearrange("(e one) -> e one", one=1)


def kernel(z, Wr, br, W1, b1, W2, b2, gamma, beta, Wo, bo):
    z = np.ascontiguousarray(np.asarray(z, dtype=np.float32))
    Wr = np.asarray(Wr, dtype=np.float32)
    br = np.asarray(br, dtype=np.float32)
    W1 = np.asarray(W1, dtype=np.float32)
    b1 = np.asarray(b1, dtype=np.float32)
    W2 = np.asarray(W2, dtype=np.float32)
    b2 = np.asarray(b2, dtype=np.float32)
    gamma = np.asarray(gamma, dtype=np.float32)
    beta = np.asarray(beta, dtype=np.float32)
    Wo = np.asarray(Wo, dtype=np.float32)
    bo = np.asarray(bo, dtype=np.float32)

    # ---- host routing replica (control plane: packing only) ----
    logits = z @ Wr + br
    m = logits.max(-1, keepdims=True)
    ex = np.exp((logits - m) / TAU)
    probs_h = ex / ex.sum(-1, keepdims=True)
    order = np.argsort(-probs_h, axis=-1, kind="stable")
    top2 = order[:, :2]

    counts = np.zeros((NCORES, E), dtype=np.int64)
    for c in range(NCORES):
        t2 = top2[c * NTOK : (c + 1) * NTOK]
        for e in range(E):
            counts[c, e] = (t2 == e).sum()
    C = int(((counts.max() + 128 + 127) // 128) * 128)

    # packed layouts + slot maps
    zT = np.ascontiguousarray(z.T)  # [D, N]
    gwo = (gamma * Wo[:, :, 0])  # [E, H]
    statlhs = np.stack([np.ones((E, D), np.float32), gwo], axis=-1).astype(np.float32)
    swo = gwo.sum(-1).astype(np.float32)
    bop = (bo[:, 0] + (beta * Wo[:, :, 0]).sum(-1)).astype(np.float32)

    # pre-tiled weight layouts (match SBUF tile layout -> contiguous DMAs)
    RG, NG = 4, TT // 4
    w1_t = np.ascontiguousarray(W1.reshape(E, KC, P, H).transpose(0, 2, 1, 3))
    w2_t = np.ascontiguousarray(W2.reshape(E, KC, P, H).transpose(0, 2, 1, 3))
    sl_t = np.ascontiguousarray(statlhs.reshape(E, KC, P, 2).transpose(0, 2, 1, 3))
    b1_t = np.ascontiguousarray(b1.reshape(E, H // P, P).transpose(0, 2, 1))
    b2_t = np.ascontiguousarray(b2.reshape(E, H // P, P).transpose(0, 2, 1))

    def _pack_blocks(zTp):
        Zp = zTp.reshape(KC, P, E * C)
        blocks = []
        for e in range(E):
            for (c0, tw) in _chunks(C):
                blocks.append(
                    Zp[:, :, e * C + c0 : e * C + c0 + tw].transpose(1, 0, 2).reshape(-1)
                )
        return np.concatenate(blocks)

    in_maps = []
    slot_of = np.zeros((N, 2), dtype=np.int64)  # global slot per (token, k)
    for c in range(NCORES):
        lo = c * NTOK
        t2 = top2[lo : lo + NTOK]
        zTc = zT[:, lo : lo + NTOK]
        zrt = np.ascontiguousarray(
            zTc.reshape(KC, P, NG, RG * P).transpose(2, 1, 0, 3)
        )
        zTp = np.zeros((D, E * C), dtype=np.float32)
        for e in range(E):
            rows, ks = np.nonzero(t2 == e)
            zTp[:, e * C : e * C + rows.size] = zTc[:, rows]
            slot_of[lo + rows, ks] = c * (E * C) + e * C + np.arange(rows.size)
        in_maps.append({
            "zrt": zrt, "zTp": _pack_blocks(zTp), "w1": w1_t, "w2": w2_t,
            "wr": Wr, "br": br, "b1": b1_t, "b2": b2_t, "statlhs": sl_t,
            "swo": swo, "bop": bop,
        })

    key = ("nc", C)
    if key not in _cache:
        _cache[key] = build_kernel(C)
    nc = _cache[key]
    global _last_in_maps
    _last_in_maps = in_maps

    res = run_bass_kernel_spmd(nc, in_maps, core_ids=list(range(NCORES)))

    probs = np.concatenate([r["probs"] for r in res.results], axis=0)
    topk_idx = np.concatenate([r["topk"] for r in res.results], axis=0).astype(np.int32)
    gates = np.concatenate([r["gates"] for r in res.results], axis=0)
    ye = np.concatenate([r["ye"].reshape(-1) for r in res.results], axis=0)  # [NCORES*E*C]
    il = np.stack([r["implood"][0].reshape(TT, 2 * E).sum(0) for r in res.results], axis=0)

    y_hat = (gates[:, 0] * ye[slot_of[:, 0]] + gates[:, 1] * ye[slot_of[:, 1]])
    y_hat = y_hat.astype(np.float32)[:, None]
    importance = (il[:, :E].sum(0) / N).astype(np.float32)
    load = (il[:, E:].sum(0) / N).astype(np.float32)
    return y_hat, probs, topk_idx, importance, load


# revision 16
# speedup vs baseline: 1.2125x; 1.1330x over previous
"""MoE head (router + top-2 gated 8-expert ResidualMLP) on 8 Trainium2 cores.

Strategy: tokens are sharded across the 8 cores (4096 each); the router and
all weights are replicated. Top-2 sparsity is exploited by packing each
core's tokens per expert on the host (control-plane only — all tensor math
runs on device): for each core we build zT_packed[D, E*C] whose column block
e holds the (z-transposed) tokens routed to expert e, padded to a fixed
capacity C. The device computes, per core:
  - fp32 router: logits -> softmax(probs) -> top-2 (values+indices) -> gates
  - per-expert MLP on the packed tokens in fp32r (1 cyc/row on the PE):
      h1 = gelu(z @ W1[e] + b1[e]); x = h1 @ W2[e] + b2[e] + z
    and the layernorm+head collapsed to three per-token reductions
    (sum(x), sum(x^2), u = x . (gamma*Wo)) done as M<=3 matmuls, so
      ye = (u - mu * sum(gamma*Wo)) * rsqrt(var + eps) + (bo + beta . Wo)
  - partial sums over tokens for importance/load (ones-vector matmul)
The host combines: y[t] = w2[t,0]*ye[slot(t, top1)] + w2[t,1]*ye[slot(t, top2)],
concatenates the token-sharded outputs, and reduces the importance/load
partials. Dropping is impossible by construction: C is chosen at runtime
from the host routing replica with >=128 slack.
"""

import sys

import numpy as np

sys.path.insert(0, "/opt/trn_rl_repo")

import concourse.bass as bass  # noqa: E402
import concourse.mybir as mybir  # noqa: E402
from concourse import bacc  # noqa: E402
from concourse.tile import TileContext  # noqa: E402
from concourse.bass_utils import run_bass_kernel_spmd  # noqa: E402

N, D, H, E, TOPK, OUT = 32768, 1024, 1024, 8, 2, 1
TAU = 1.2
LN_EPS = 1e-5
NCORES = 8
NTOK = N // NCORES           # tokens per core
P = 128                      # partitions
KC = D // P                  # contraction chunks (8)
HT = H // P                  # output row tiles (8)
TT = NTOK // P               # router token tiles per core (32)
RG = 4                       # router token tiles batched per group
F32 = mybir.dt.float32
F32R = mybir.dt.float32r
U32 = mybir.dt.uint32

_cache = {}


def _chunks(c):
    out = []
    s = 0
    while s < c:
        t = min(512, c - s)
        out.append((s, t))
        s += t
    if len(out) >= 2 and out[-1][1] == 128:
        # avoid a 128-wide tail (fp32r needs >=256 rows for full rate)
        (s0, t0), (s1, _) = out[-2], out[-1]
        out[-2] = (s0, 384)
        out[-1] = (s0 + 384, 256)
    return out


def build_kernel(C):
    nc = bacc.Bacc("TRN2", target_bir_lowering=False, debug=False)
    S = E * C

    # inputs (per core)
    NG = TT // RG
    zrt_d = nc.dram_tensor("zrt", [NG, P, KC, RG * P], F32, kind="ExternalInput")
    zTp_d = nc.dram_tensor("zTp", [D * S], F32R, kind="ExternalInput")
    w1_d = nc.dram_tensor("w1", [E, P, KC, H], F32R, kind="ExternalInput")
    w2_d = nc.dram_tensor("w2", [E, P, KC, H], F32R, kind="ExternalInput")
    wr_d = nc.dram_tensor("wr", [D, E], F32, kind="ExternalInput")
    br_d = nc.dram_tensor("br", [E], F32, kind="ExternalInput")
    b1_d = nc.dram_tensor("b1", [E, P, HT], F32, kind="ExternalInput")
    b2_d = nc.dram_tensor("b2", [E, P, HT], F32, kind="ExternalInput")
    # statlhs[e,:,0] = 1, statlhs[e,:,1] = (gamma[e]*Wo[e,:,0])
    sl_d = nc.dram_tensor("statlhs", [E, P, KC, 2], F32R, kind="ExternalInput")
    # per-expert scalars: swo[e] = sum(gamma[e]*Wo[e]), bop[e] = bo[e]+beta[e].Wo[e]
    swo_d = nc.dram_tensor("swo", [E], F32, kind="ExternalInput")
    bop_d = nc.dram_tensor("bop", [E], F32, kind="ExternalInput")

    # outputs (per core)
    probs_o = nc.dram_tensor("probs", [NTOK, E], F32, kind="ExternalOutput")
    idx_o = nc.dram_tensor("topk", [NTOK, 2], mybir.dt.int32, kind="ExternalOutput")
    w2g_o = nc.dram_tensor("gates", [NTOK, 2], F32, kind="ExternalOutput")
    ye_o = nc.dram_tensor("ye", [E, C], F32, kind="ExternalOutput")
    il_o = nc.dram_tensor("implood", [1, TT * 2 * E], F32, kind="ExternalOutput")

    with TileContext(nc) as tc:
        with tc.tile_pool(name="consts", bufs=1) as consts, \
             tc.tile_pool(name="wpool", bufs=1) as wpool, \
             tc.tile_pool(name="act", bufs=2) as act, \
             tc.tile_pool(name="small", bufs=3) as small, \
             tc.tile_pool(name="rt", bufs=2) as rt, \
             tc.tile_pool(name="stats", bufs=1) as stats, \
             tc.tile_pool(name="ps_h1", bufs=2, space="PSUM") as ps_h1, \
             tc.tile_pool(name="ps_h2", bufs=2, space="PSUM") as ps_h2, \
             tc.tile_pool(name="ps_st", bufs=1, space="PSUM") as ps_st, \
             tc.tile_pool(name="ps_sq", bufs=1, space="PSUM") as ps_sq, \
             tc.tile_pool(name="ps_rt", bufs=1, space="PSUM") as ps_rt, \
             tc.tile_pool(name="ps_misc", bufs=1, space="PSUM") as ps_misc:

            # ---------- constants ----------
            wr_t = consts.tile([P, KC, E], F32)
            nc.sync.dma_start(out=wr_t, in_=wr_d.rearrange("(k p) e -> p k e", p=P))
            br_b = consts.tile([P, E], F32)
            nc.sync.dma_start(out=br_b, in_=bass.AP(tensor=br_d, offset=0, ap=[[0, P], [1, E]]))
            swo_t = consts.tile([E, 1], F32)
            nc.sync.dma_start(out=swo_t, in_=sl_wrap(swo_d))
            bop_t = consts.tile([E, 1], F32)
            nc.sync.dma_start(out=bop_t, in_=sl_wrap(bop_d))

            # stats accumulators [E, C] (partition = expert)
            sumx_all = stats.tile([E, C], F32)
            u_all = stats.tile([E, C], F32)
            sumsq_all = stats.tile([E, C], F32)

            ones1 = consts.tile([P, 1], F32)
            nc.vector.memset(ones1, 1.0)
            pm_keep = consts.tile([P, TT, 2 * E], F32)

            # ---------- router (token-major fp32: bit-stable vs reference) ----------
            n_groups = TT // RG
            for g in range(n_groups):
                g0 = g * RG
                gn = RG
                gP = gn * P
                ztile = rt.tile([P, KC, RG * P], F32, tag="zrt")
                nc.gpsimd.dma_start(out=ztile, in_=zrt_d[g])
                lg_sb = rt.tile([P, RG, E], F32, tag="lg")
                for gi in range(gn):
                    lg_ps = ps_rt.tile([P, E], F32, tag="lgps")
                    for k in range(KC):
                        nc.tensor.matmul(
                            lg_ps,
                            ztile[:, k, gi * P : (gi + 1) * P],
                            wr_t[:, k, :],
                            start=(k == 0),
                            stop=(k == KC - 1),
                        )
                    nc.vector.tensor_add(lg_sb[:, gi, :], lg_ps, br_b)
                # softmax over E for the whole group
                mx = rt.tile([P, RG, 1], F32, tag="mx")
                nc.vector.reduce_max(mx[:, :gn, :], lg_sb[:, :gn, :], axis=mybir.AxisListType.X)
                bm = rt.tile([P, RG, 1], F32, tag="bm")
                nc.vector.tensor_scalar_mul(bm[:, :gn, :], mx[:, :gn, :], -1.0 / TAU)
                ex = rt.tile([P, RG, E], F32, tag="ex")
                for gi in range(gn):
                    nc.scalar.activation(
                        ex[:, gi, :], lg_sb[:, gi, :],
                        mybir.ActivationFunctionType.Exp,
                        bias=bm[:, gi, :], scale=1.0 / TAU,
                    )
                sm = rt.tile([P, RG, 1], F32, tag="sm")
                nc.vector.reduce_sum(sm[:, :gn, :], ex[:, :gn, :], axis=mybir.AxisListType.X)
                rs = rt.tile([P, RG, 1], F32, tag="rs")
                nc.vector.reciprocal(rs[:, :gn, :], sm[:, :gn, :])
                probs = rt.tile([P, RG, E], F32, tag="probs")
                mask = rt.tile([P, RG, E], F32, tag="mask")
                vals = rt.tile([P, RG, 8], F32, tag="vals")
                idxs = rt.tile([P, RG, 8], U32, tag="idxs")
                den = rt.tile([P, RG, 1], F32, tag="den")
                w2g = rt.tile([P, RG, 2], F32, tag="w2g")
                for gi in range(gn):
                    nc.vector.tensor_scalar(
                        probs[:, gi, :], ex[:, gi, :], rs[:, gi, :],
                        scalar2=None, op0=mybir.AluOpType.mult,
                    )
                    nc.vector.max_with_indices(vals[:, gi, :], idxs[:, gi, :], probs[:, gi, :])
                    nc.vector.tensor_add(den[:, gi, :], vals[:, gi, 0:1], vals[:, gi, 1:2])
                    nc.vector.tensor_scalar_max(den[:, gi, :], den[:, gi, :], 1e-8)
                    nc.vector.reciprocal(den[:, gi, :], den[:, gi, :])
                    nc.vector.tensor_scalar(
                        w2g[:, gi, :], vals[:, gi, 0:2], den[:, gi, :],
                        scalar2=None, op0=mybir.AluOpType.mult,
                    )
                    nc.vector.tensor_scalar(
                        mask[:, gi, :], probs[:, gi, :], vals[:, gi, 1:2],
                        scalar2=None, op0=mybir.AluOpType.is_ge,
                    )
                    nc.vector.tensor_copy(pm_keep[:, g0 + gi, :E], probs[:, gi, :])
                    nc.vector.tensor_copy(pm_keep[:, g0 + gi, E:], mask[:, gi, :])
                pr_v = probs_o.rearrange("(g t) e -> t g e", t=P)
                nc.sync.dma_start(out=pr_v[:, g0 : g0 + gn, :], in_=probs[:, :gn, :])
                ix_v = idx_o.rearrange("(g t) e -> t g e", t=P)
                nc.gpsimd.dma_start(out=ix_v[:, g0 : g0 + gn, :], in_=idxs[:, :gn, 0:2])
                wg_v = w2g_o.rearrange("(g t) e -> t g e", t=P)
                nc.sync.dma_start(out=wg_v[:, g0 : g0 + gn, :], in_=w2g[:, :gn, :])

            # ---------- experts on packed tokens (fp32r) ----------

            for e in range(E):
                w1t = wpool.tile([P, KC, H], F32R, tag="w1")
                nc.sync.dma_start(out=w1t, in_=w1_d[e])
                w2t = wpool.tile([P, KC, H], F32R, tag="w2")
                nc.sync.dma_start(out=w2t, in_=w2_d[e])
                b1t = small.tile([P, HT], F32, tag="b1")
                nc.sync.dma_start(out=b1t, in_=b1_d[e])
                b2t = small.tile([P, HT], F32, tag="b2")
                nc.sync.dma_start(out=b2t, in_=b2_d[e])
                slt = small.tile([P, KC, 2], F32R, tag="sl")
                nc.sync.dma_start(out=slt, in_=sl_d[e])

                for (c0, tw) in _chunks(C):
                    zp = act.tile([P, KC, 512], F32R, tag="zp")
                    off = (e * C + c0) * D
                    nc.sync.dma_start(
                        out=zp[:, :, :tw],
                        in_=zTp_d[off : off + P * KC * tw].rearrange(
                            "(p k t) -> p k t", p=P, k=KC
                        ),
                    )
                    h1g = act.tile([P, KC, 512], F32R, tag="h1g")
                    for ht in range(HT):
                        h1_ps = ps_h1.tile([P, 512], F32, tag="h1ps")
                        for k in range(KC):
                            nc.tensor.matmul(
                                h1_ps[:, :tw],
                                w1t[:, k, ht * P : (ht + 1) * P],
                                zp[:, k, :tw],
                                start=(k == 0),
                                stop=(k == KC - 1),
                            )
                        nc.scalar.activation(
                            h1g[:, ht, :tw], h1_ps[:, :tw],
                            mybir.ActivationFunctionType.Gelu,
                            bias=b1t[:, ht : ht + 1], scale=1.0,
                        )
                    st_ps = ps_st.tile([2, 512], F32, tag="stps")
                    sq_ps = ps_sq.tile([1, 512], F32, tag="sqps")
                    for ht in range(HT):
                        h2_ps = ps_h2.tile([P, 512], F32, tag="h2ps")
                        for k in range(KC):
                            nc.tensor.matmul(
                                h2_ps[:, :tw],
                                w2t[:, k, ht * P : (ht + 1) * P],
                                h1g[:, k, :tw],
                                start=(k == 0),
                                stop=(k == KC - 1),
                            )
                        # x = h2 + b2 + z  (in place into zp[ht])
                        tmp = act.tile([P, 512], F32, tag="tmp")
                        nc.scalar.activation(
                            tmp[:, :tw], h2_ps[:, :tw],
                            mybir.ActivationFunctionType.Identity,
                            bias=b2t[:, ht : ht + 1], scale=1.0,
                        )
                        nc.vector.tensor_add(zp[:, ht, :tw], zp[:, ht, :tw], tmp[:, :tw])
                    # stats: [sumx; u] then sumsq
                    for k in range(KC):
                        nc.tensor.matmul(
                            st_ps[0:2, :tw], slt[:, k, :], zp[:, k, :tw],
                            start=(k == 0), stop=(k == KC - 1),
                        )
                    for k in range(KC):
                        x2t = act.tile([P, 512], F32R, tag="x2")
                        nc.vector.tensor_mul(x2t[:, :tw], zp[:, k, :tw], zp[:, k, :tw])
                        nc.tensor.matmul(
                            sq_ps[0:1, :tw], slt[:, k, 0:1], x2t[:, :tw],
                            start=(k == 0), stop=(k == KC - 1),
                        )
                    stg_su = act.tile([2, 512], F32, tag="stgsu")
                    stg_sq = act.tile([1, 512], F32, tag="stgsq")
                    nc.vector.tensor_copy(stg_su[:, :tw], st_ps[:, :tw])
                    nc.vector.tensor_copy(stg_sq[:, :tw], sq_ps[:, :tw])
                    nc.sync.dma_start(out=sumx_all[e : e + 1, c0 : c0 + tw], in_=stg_su[0:1, :tw])
                    nc.sync.dma_start(out=u_all[e : e + 1, c0 : c0 + tw], in_=stg_su[1:2, :tw])
                    nc.sync.dma_start(out=sumsq_all[e : e + 1, c0 : c0 + tw], in_=stg_sq[:, :tw])

            # ---------- deferred importance/load partial sums ----------
            ilg_ps = ps_misc.tile([1, TT * 2 * E], F32, tag="ilps")
            nc.tensor.matmul(ilg_ps, ones1, pm_keep, start=True, stop=True)
            il_stage = small.tile([1, TT * 2 * E], F32, tag="ilstg")
            nc.vector.tensor_copy(il_stage, ilg_ps)
            nc.sync.dma_start(out=il_o[:, :], in_=il_stage)

            # ---------- final per-slot LN + head math on [E, C] ----------
            # in-place: mu lives in sumx_all, var in sumsq_all, rstd in tmp_s
            tmp_s = stats.tile([E, C], F32)
            eps_t = stats.tile([E, 1], F32)
            nc.vector.memset(eps_t, LN_EPS)
            nc.vector.tensor_scalar_mul(sumx_all, sumx_all, 1.0 / H)   # mu
            nc.vector.tensor_scalar_mul(sumsq_all, sumsq_all, 1.0 / H) # E[x^2]
            nc.vector.tensor_mul(tmp_s, sumx_all, sumx_all)            # mu^2
            nc.vector.tensor_sub(sumsq_all, sumsq_all, tmp_s)          # var
            nc.scalar.activation(tmp_s, sumsq_all, mybir.ActivationFunctionType.Sqrt,
                                 bias=eps_t, scale=1.0)
            nc.vector.reciprocal(tmp_s, tmp_s)                         # rstd
            # ye = (u - mu*swo) * rstd + bop
            nc.vector.tensor_scalar(sumx_all, sumx_all, swo_t, scalar2=None,
                                    op0=mybir.AluOpType.mult)
            nc.vector.tensor_sub(u_all, u_all, sumx_all)
            nc.vector.tensor_mul(u_all, u_all, tmp_s)
            nc.vector.tensor_scalar(u_all, u_all, bop_t, scalar2=None,
                                    op0=mybir.AluOpType.add)
            nc.sync.dma_start(out=ye_o[:, :], in_=u_all)

    nc.compile()
    return nc


def sl_wrap(d):
    return d.rearrange("(e one) -> e one", one=1)


def kernel(z, Wr, br, W1, b1, W2, b2, gamma, beta, Wo, bo):
    z = np.ascontiguousarray(np.asarray(z, dtype=np.float32))
    Wr = np.asarray(Wr, dtype=np.float32)
    br = np.asarray(br, dtype=np.float32)
    W1 = np.asarray(W1, dtype=np.float32)
    b1 = np.asarray(b1, dtype=np.float32)
    W2 = np.asarray(W2, dtype=np.float32)
    b2 = np.asarray(b2, dtype=np.float32)
    gamma = np.asarray(gamma, dtype=np.float32)
    beta = np.asarray(beta, dtype=np.float32)
    Wo = np.asarray(Wo, dtype=np.float32)
    bo = np.asarray(bo, dtype=np.float32)

    # ---- host routing replica (control plane: packing only) ----
    logits = z @ Wr + br
    m = logits.max(-1, keepdims=True)
    ex = np.exp((logits - m) / TAU)
    probs_h = ex / ex.sum(-1, keepdims=True)
    order = np.argsort(-probs_h, axis=-1, kind="stable")
    top2 = order[:, :2]

    counts = np.zeros((NCORES, E), dtype=np.int64)
    for c in range(NCORES):
        t2 = top2[c * NTOK : (c + 1) * NTOK]
        for e in range(E):
            counts[c, e] = (t2 == e).sum()
    C = int(((counts.max() + 127) // 128) * 128)

    # packed layouts + slot maps
    zT = np.ascontiguousarray(z.T)  # [D, N]
    gwo = (gamma * Wo[:, :, 0])  # [E, H]
    statlhs = np.stack([np.ones((E, D), np.float32), gwo], axis=-1).astype(np.float32)
    swo = gwo.sum(-1).astype(np.float32)
    bop = (bo[:, 0] + (beta * Wo[:, :, 0]).sum(-1)).astype(np.float32)

    # pre-tiled weight layouts (match SBUF tile layout -> contiguous DMAs)
    RG, NG = 4, TT // 4
    w1_t = np.ascontiguousarray(W1.reshape(E, KC, P, H).transpose(0, 2, 1, 3))
    w2_t = np.ascontiguousarray(W2.reshape(E, KC, P, H).transpose(0, 2, 1, 3))
    sl_t = np.ascontiguousarray(statlhs.reshape(E, KC, P, 2).transpose(0, 2, 1, 3))
    b1_t = np.ascontiguousarray(b1.reshape(E, H // P, P).transpose(0, 2, 1))
    b2_t = np.ascontiguousarray(b2.reshape(E, H // P, P).transpose(0, 2, 1))

    def _pack_blocks(zTp):
        Zp = zTp.reshape(KC, P, E * C)
        blocks = []
        for e in range(E):
            for (c0, tw) in _chunks(C):
                blocks.append(
                    Zp[:, :, e * C + c0 : e * C + c0 + tw].transpose(1, 0, 2).reshape(-1)
                )
        return np.concatenate(blocks)

    in_maps = []
    slot_of = np.zeros((N, 2), dtype=np.int64)  # global slot per (token, k)
    for c in range(NCORES):
        lo = c * NTOK
        t2 = top2[lo : lo + NTOK]
        zTc = zT[:, lo : lo + NTOK]
        zrt = np.ascontiguousarray(
            zTc.reshape(KC, P, NG, RG * P).transpose(2, 1, 0, 3)
        )
        zTp = np.zeros((D, E * C), dtype=np.float32)
        for e in range(E):
            rows, ks = np.nonzero(t2 == e)
            zTp[:, e * C : e * C + rows.size] = zTc[:, rows]
            slot_of[lo + rows, ks] = c * (E * C) + e * C + np.arange(rows.size)
        in_maps.append({
            "zrt": zrt, "zTp": _pack_blocks(zTp), "w1": w1_t, "w2": w2_t,
            "wr": Wr, "br": br, "b1": b1_t, "b2": b2_t, "statlhs": sl_t,
            "swo": swo, "bop": bop,
        })

    key = ("nc", C)
    if key not in _cache:
        _cache[key] = build_kernel(C)
    nc = _cache[key]
    global _last_in_maps
    _last_in_maps = in_maps

    res = run_bass_kernel_spmd(nc, in_maps, core_ids=list(range(NCORES)))

    probs = np.concatenate([r["probs"] for r in res.results], axis=0)
    topk_idx = np.concatenate([r["topk"] for r in res.results], axis=0).astype(np.int32)
    gates = np.concatenate([r["gates"] for r in res.results], axis=0)
    ye = np.concatenate([r["ye"].reshape(-1) for r in res.results], axis=0)  # [NCORES*E*C]
    il = np.stack([r["implood"][0].reshape(TT, 2 * E).sum(0) for r in res.results], axis=0)

    y_hat = (gates[:, 0] * ye[slot_of[:, 0]] + gates[:, 1] * ye[slot_of[:, 1]])
    y_hat = y_hat.astype(np.float32)[:, None]
    importance = (il[:, :E].sum(0) / N).astype(np.float32)
    load = (il[:, E:].sum(0) / N).astype(np.float32)
    return y_hat, probs, topk_idx, importance, load


# revision 17
# speedup vs baseline: 1.2714x; 1.0486x over previous
"""MoE head (router + top-2 gated 8-expert ResidualMLP) on 8 Trainium2 cores.

Strategy: tokens are sharded across the 8 cores (4096 each); the router and
all weights are replicated. Top-2 sparsity is exploited by packing each
core's tokens per expert on the host (control-plane only — all tensor math
runs on device): for each core we build zT_packed[D, E*C] whose column block
e holds the (z-transposed) tokens routed to expert e, padded to a fixed
capacity C. The device computes, per core:
  - fp32 router: logits -> softmax(probs) -> top-2 (values+indices) -> gates
  - per-expert MLP on the packed tokens in fp32r (1 cyc/row on the PE):
      h1 = gelu(z @ W1[e] + b1[e]); x = h1 @ W2[e] + b2[e] + z
    and the layernorm+head collapsed to three per-token reductions
    (sum(x), sum(x^2), u = x . (gamma*Wo)) done as M<=3 matmuls, so
      ye = (u - mu * sum(gamma*Wo)) * rsqrt(var + eps) + (bo + beta . Wo)
  - partial sums over tokens for importance/load (ones-vector matmul)
The host combines: y[t] = w2[t,0]*ye[slot(t, top1)] + w2[t,1]*ye[slot(t, top2)],
concatenates the token-sharded outputs, and reduces the importance/load
partials. Dropping is impossible by construction: C is chosen at runtime
from the host routing replica with >=128 slack.
"""

import sys

import numpy as np

sys.path.insert(0, "/opt/trn_rl_repo")

import concourse.bass as bass  # noqa: E402
import concourse.mybir as mybir  # noqa: E402
from concourse import bacc  # noqa: E402
from concourse.tile import TileContext  # noqa: E402
from concourse.bass_utils import run_bass_kernel_spmd  # noqa: E402

N, D, H, E, TOPK, OUT = 32768, 1024, 1024, 8, 2, 1
TAU = 1.2
LN_EPS = 1e-5
NCORES = 8
NTOK = N // NCORES           # tokens per core
P = 128                      # partitions
KC = D // P                  # contraction chunks (8)
HT = H // P                  # output row tiles (8)
TT = NTOK // P               # router token tiles per core (32)
RG = 2                       # router token tiles batched per group
F32 = mybir.dt.float32
F32R = mybir.dt.float32r
U32 = mybir.dt.uint32

_cache = {}


def _chunks(c):
    out = []
    s = 0
    while s < c:
        t = min(512, c - s)
        out.append((s, t))
        s += t
    if len(out) >= 2 and out[-1][1] == 128:
        # avoid a 128-wide tail (fp32r needs >=256 rows for full rate)
        (s0, t0), (s1, _) = out[-2], out[-1]
        out[-2] = (s0, 384)
        out[-1] = (s0 + 384, 256)
    return out


def build_kernel(C):
    nc = bacc.Bacc("TRN2", target_bir_lowering=False, debug=False)
    S = E * C

    # inputs (per core)
    NG = TT // RG
    zrt_d = nc.dram_tensor("zrt", [NG, P, KC, RG * P], F32, kind="ExternalInput")
    zTp_d = nc.dram_tensor("zTp", [D * S], F32R, kind="ExternalInput")
    w1_d = nc.dram_tensor("w1", [E, P, KC, H], F32R, kind="ExternalInput")
    w2_d = nc.dram_tensor("w2", [E, P, KC, H], F32R, kind="ExternalInput")
    wr_d = nc.dram_tensor("wr", [D, E], F32, kind="ExternalInput")
    br_d = nc.dram_tensor("br", [E], F32, kind="ExternalInput")
    b1_d = nc.dram_tensor("b1", [E, P, HT], F32, kind="ExternalInput")
    b2_d = nc.dram_tensor("b2", [E, P, HT], F32, kind="ExternalInput")
    # statlhs[e,:,0] = 1, statlhs[e,:,1] = (gamma[e]*Wo[e,:,0])
    sl_d = nc.dram_tensor("statlhs", [E, P, KC, 2], F32R, kind="ExternalInput")
    # per-expert scalars: swo[e] = sum(gamma[e]*Wo[e]), bop[e] = bo[e]+beta[e].Wo[e]
    swo_d = nc.dram_tensor("swo", [E], F32, kind="ExternalInput")
    bop_d = nc.dram_tensor("bop", [E], F32, kind="ExternalInput")

    # outputs (per core)
    probs_o = nc.dram_tensor("probs", [NTOK, E], F32, kind="ExternalOutput")
    idx_o = nc.dram_tensor("topk", [NTOK, 2], mybir.dt.int32, kind="ExternalOutput")
    w2g_o = nc.dram_tensor("gates", [NTOK, 2], F32, kind="ExternalOutput")
    ye_o = nc.dram_tensor("ye", [E, C], F32, kind="ExternalOutput")
    il_o = nc.dram_tensor("implood", [1, TT * 2 * E], F32, kind="ExternalOutput")

    with TileContext(nc) as tc:
        with tc.tile_pool(name="consts", bufs=1) as consts, \
             tc.tile_pool(name="wpool", bufs=1) as wpool, \
             tc.tile_pool(name="act", bufs=2) as act, \
             tc.tile_pool(name="small", bufs=3) as small, \
             tc.tile_pool(name="rt", bufs=2) as rt, \
             tc.tile_pool(name="stats", bufs=1) as stats, \
             tc.tile_pool(name="ps_h1", bufs=2, space="PSUM") as ps_h1, \
             tc.tile_pool(name="ps_h2", bufs=2, space="PSUM") as ps_h2, \
             tc.tile_pool(name="ps_st", bufs=1, space="PSUM") as ps_st, \
             tc.tile_pool(name="ps_sq", bufs=1, space="PSUM") as ps_sq, \
             tc.tile_pool(name="ps_rt", bufs=1, space="PSUM") as ps_rt, \
             tc.tile_pool(name="ps_misc", bufs=1, space="PSUM") as ps_misc:

            # ---------- constants ----------
            wr_t = consts.tile([P, KC, E], F32)
            nc.sync.dma_start(out=wr_t, in_=wr_d.rearrange("(k p) e -> p k e", p=P))
            br_b = consts.tile([P, E], F32)
            nc.sync.dma_start(out=br_b, in_=bass.AP(tensor=br_d, offset=0, ap=[[0, P], [1, E]]))
            swo_t = consts.tile([E, 1], F32)
            nc.sync.dma_start(out=swo_t, in_=sl_wrap(swo_d))
            bop_t = consts.tile([E, 1], F32)
            nc.sync.dma_start(out=bop_t, in_=sl_wrap(bop_d))

            # stats accumulators [E, C] (partition = expert)
            sumx_all = stats.tile([E, C], F32)
            u_all = stats.tile([E, C], F32)
            sumsq_all = stats.tile([E, C], F32)

            ones1 = consts.tile([P, 1], F32)
            nc.vector.memset(ones1, 1.0)
            pm_keep = consts.tile([P, TT, 2 * E], F32)

            # ---------- router (token-major fp32: bit-stable vs reference) ----------
            n_groups = TT // RG
            for g in range(n_groups):
                g0 = g * RG
                gn = RG
                gP = gn * P
                ztile = rt.tile([P, KC, RG * P], F32, tag="zrt")
                nc.gpsimd.dma_start(out=ztile, in_=zrt_d[g])
                lg_sb = rt.tile([P, RG, E], F32, tag="lg")
                for gi in range(gn):
                    lg_ps = ps_rt.tile([P, E], F32, tag="lgps")
                    for k in range(KC):
                        nc.tensor.matmul(
                            lg_ps,
                            ztile[:, k, gi * P : (gi + 1) * P],
                            wr_t[:, k, :],
                            start=(k == 0),
                            stop=(k == KC - 1),
                        )
                    nc.vector.tensor_add(lg_sb[:, gi, :], lg_ps, br_b)
                # softmax over E for the whole group
                mx = rt.tile([P, RG, 1], F32, tag="mx")
                nc.vector.reduce_max(mx[:, :gn, :], lg_sb[:, :gn, :], axis=mybir.AxisListType.X)
                bm = rt.tile([P, RG, 1], F32, tag="bm")
                nc.vector.tensor_scalar_mul(bm[:, :gn, :], mx[:, :gn, :], -1.0 / TAU)
                ex = rt.tile([P, RG, E], F32, tag="ex")
                for gi in range(gn):
                    nc.scalar.activation(
                        ex[:, gi, :], lg_sb[:, gi, :],
                        mybir.ActivationFunctionType.Exp,
                        bias=bm[:, gi, :], scale=1.0 / TAU,
                    )
                sm = rt.tile([P, RG, 1], F32, tag="sm")
                nc.vector.reduce_sum(sm[:, :gn, :], ex[:, :gn, :], axis=mybir.AxisListType.X)
                rs = rt.tile([P, RG, 1], F32, tag="rs")
                nc.vector.reciprocal(rs[:, :gn, :], sm[:, :gn, :])
                probs = rt.tile([P, RG, E], F32, tag="probs")
                mask = rt.tile([P, RG, E], F32, tag="mask")
                vals = rt.tile([P, RG, 8], F32, tag="vals")
                idxs = rt.tile([P, RG, 8], U32, tag="idxs")
                den = rt.tile([P, RG, 1], F32, tag="den")
                w2g = rt.tile([P, RG, 2], F32, tag="w2g")
                for gi in range(gn):
                    nc.vector.tensor_scalar(
                        probs[:, gi, :], ex[:, gi, :], rs[:, gi, :],
                        scalar2=None, op0=mybir.AluOpType.mult,
                    )
                    nc.vector.max_with_indices(vals[:, gi, :], idxs[:, gi, :], probs[:, gi, :])
                    nc.vector.tensor_add(den[:, gi, :], vals[:, gi, 0:1], vals[:, gi, 1:2])
                    nc.vector.tensor_scalar_max(den[:, gi, :], den[:, gi, :], 1e-8)
                    nc.vector.reciprocal(den[:, gi, :], den[:, gi, :])
                    nc.vector.tensor_scalar(
                        w2g[:, gi, :], vals[:, gi, 0:2], den[:, gi, :],
                        scalar2=None, op0=mybir.AluOpType.mult,
                    )
                    nc.vector.tensor_scalar(
                        mask[:, gi, :], probs[:, gi, :], vals[:, gi, 1:2],
                        scalar2=None, op0=mybir.AluOpType.is_ge,
                    )
                    nc.vector.tensor_copy(pm_keep[:, g0 + gi, :E], probs[:, gi, :])
                    nc.vector.tensor_copy(pm_keep[:, g0 + gi, E:], mask[:, gi, :])
                pr_v = probs_o.rearrange("(g t) e -> t g e", t=P)
                nc.sync.dma_start(out=pr_v[:, g0 : g0 + gn, :], in_=probs[:, :gn, :])
                ix_v = idx_o.rearrange("(g t) e -> t g e", t=P)
                nc.gpsimd.dma_start(out=ix_v[:, g0 : g0 + gn, :], in_=idxs[:, :gn, 0:2])
                wg_v = w2g_o.rearrange("(g t) e -> t g e", t=P)
                nc.sync.dma_start(out=wg_v[:, g0 : g0 + gn, :], in_=w2g[:, :gn, :])

            # ---------- experts on packed tokens (fp32r) ----------

            for e in range(E):
                w1t = wpool.tile([P, KC, H], F32R, tag="w1")
                nc.sync.dma_start(out=w1t[:, :, : H // 2], in_=w1_d[e, :, :, : H // 2])
                nc.sync.dma_start(out=w1t[:, :, H // 2 :], in_=w1_d[e, :, :, H // 2 :])
                w2t = wpool.tile([P, KC, H], F32R, tag="w2")
                nc.sync.dma_start(out=w2t[:, :, : H // 2], in_=w2_d[e, :, :, : H // 2])
                nc.sync.dma_start(out=w2t[:, :, H // 2 :], in_=w2_d[e, :, :, H // 2 :])
                b1t = small.tile([P, HT], F32, tag="b1")
                nc.sync.dma_start(out=b1t, in_=b1_d[e])
                b2t = small.tile([P, HT], F32, tag="b2")
                nc.sync.dma_start(out=b2t, in_=b2_d[e])
                slt = small.tile([P, KC, 2], F32R, tag="sl")
                nc.sync.dma_start(out=slt, in_=sl_d[e])

                for (c0, tw) in _chunks(C):
                    zp = act.tile([P, KC, 512], F32R, tag="zp")
                    off = (e * C + c0) * D
                    nc.sync.dma_start(
                        out=zp[:, :, :tw],
                        in_=zTp_d[off : off + P * KC * tw].rearrange(
                            "(p k t) -> p k t", p=P, k=KC
                        ),
                    )
                    h1g = act.tile([P, KC, 512], F32R, tag="h1g")
                    for ht in range(HT):
                        h1_ps = ps_h1.tile([P, 512], F32, tag="h1ps")
                        for k in range(KC):
                            nc.tensor.matmul(
                                h1_ps[:, :tw],
                                w1t[:, k, ht * P : (ht + 1) * P],
                                zp[:, k, :tw],
                                start=(k == 0),
                                stop=(k == KC - 1),
                            )
                        nc.scalar.activation(
                            h1g[:, ht, :tw], h1_ps[:, :tw],
                            mybir.ActivationFunctionType.Gelu,
                            bias=b1t[:, ht : ht + 1], scale=1.0,
                        )
                    st_ps = ps_st.tile([2, 512], F32, tag="stps")
                    sq_ps = ps_sq.tile([1, 512], F32, tag="sqps")
                    for ht in range(HT):
                        h2_ps = ps_h2.tile([P, 512], F32, tag="h2ps")
                        for k in range(KC):
                            nc.tensor.matmul(
                                h2_ps[:, :tw],
                                w2t[:, k, ht * P : (ht + 1) * P],
                                h1g[:, k, :tw],
                                start=(k == 0),
                                stop=(k == KC - 1),
                            )
                        # x = h2 + b2 + z  (in place into zp[ht])
                        tmp = act.tile([P, 512], F32, tag="tmp")
                        nc.scalar.activation(
                            tmp[:, :tw], h2_ps[:, :tw],
                            mybir.ActivationFunctionType.Identity,
                            bias=b2t[:, ht : ht + 1], scale=1.0,
                        )
                        nc.vector.tensor_add(zp[:, ht, :tw], zp[:, ht, :tw], tmp[:, :tw])
                    # stats: [sumx; u] then sumsq
                    for k in range(KC):
                        nc.tensor.matmul(
                            st_ps[0:2, :tw], slt[:, k, :], zp[:, k, :tw],
                            start=(k == 0), stop=(k == KC - 1),
                        )
                    for k in range(KC):
                        x2t = act.tile([P, 512], F32R, tag="x2")
                        nc.vector.tensor_mul(x2t[:, :tw], zp[:, k, :tw], zp[:, k, :tw])
                        nc.tensor.matmul(
                            sq_ps[0:1, :tw], slt[:, k, 0:1], x2t[:, :tw],
                            start=(k == 0), stop=(k == KC - 1),
                        )
                    stg_su = act.tile([2, 512], F32, tag="stgsu")
                    stg_sq = act.tile([1, 512], F32, tag="stgsq")
                    nc.vector.tensor_copy(stg_su[:, :tw], st_ps[:, :tw])
                    nc.vector.tensor_copy(stg_sq[:, :tw], sq_ps[:, :tw])
                    nc.sync.dma_start(out=sumx_all[e : e + 1, c0 : c0 + tw], in_=stg_su[0:1, :tw])
                    nc.sync.dma_start(out=u_all[e : e + 1, c0 : c0 + tw], in_=stg_su[1:2, :tw])
                    nc.sync.dma_start(out=sumsq_all[e : e + 1, c0 : c0 + tw], in_=stg_sq[:, :tw])

            # ---------- deferred importance/load partial sums ----------
            ilg_ps = ps_misc.tile([1, TT * 2 * E], F32, tag="ilps")
            nc.tensor.matmul(ilg_ps, ones1, pm_keep, start=True, stop=True)
            il_stage = small.tile([1, TT * 2 * E], F32, tag="ilstg")
            nc.vector.tensor_copy(il_stage, ilg_ps)
            nc.sync.dma_start(out=il_o[:, :], in_=il_stage)

            # ---------- final per-slot LN + head math on [E, C] ----------
            # in-place: mu lives in sumx_all, var in sumsq_all, rstd in tmp_s
            tmp_s = stats.tile([E, C], F32)
            eps_t = stats.tile([E, 1], F32)
            nc.vector.memset(eps_t, LN_EPS)
            nc.vector.tensor_scalar_mul(sumx_all, sumx_all, 1.0 / H)   # mu
            nc.vector.tensor_scalar_mul(sumsq_all, sumsq_all, 1.0 / H) # E[x^2]
            nc.vector.tensor_mul(tmp_s, sumx_all, sumx_all)            # mu^2
            nc.vector.tensor_sub(sumsq_all, sumsq_all, tmp_s)          # var
            nc.scalar.activation(tmp_s, sumsq_all, mybir.ActivationFunctionType.Sqrt,
                                 bias=eps_t, scale=1.0)
            nc.vector.reciprocal(tmp_s, tmp_s)                         # rstd
            # ye = (u - mu*swo) * rstd + bop
            nc.vector.tensor_scalar(sumx_all, sumx_all, swo_t, scalar2=None,
                                    op0=mybir.AluOpType.mult)
            nc.vector.tensor_sub(u_all, u_all, sumx_all)
            nc.vector.tensor_mul(u_all, u_all, tmp_s)
            nc.vector.tensor_scalar(u_all, u_all, bop_t, scalar2=None,
                                    op0=mybir.AluOpType.add)
            nc.sync.dma_start(out=ye_o[:, :], in_=u_all)

    nc.compile()
    return nc


def sl_wrap(d):
    return d.rearrange("(e one) -> e one", one=1)


def kernel(z, Wr, br, W1, b1, W2, b2, gamma, beta, Wo, bo):
    z = np.ascontiguousarray(np.asarray(z, dtype=np.float32))
    Wr = np.asarray(Wr, dtype=np.float32)
    br = np.asarray(br, dtype=np.float32)
    W1 = np.asarray(W1, dtype=np.float32)
    b1 = np.asarray(b1, dtype=np.float32)
    W2 = np.asarray(W2, dtype=np.float32)
    b2 = np.asarray(b2, dtype=np.float32)
    gamma = np.asarray(gamma, dtype=np.float32)
    beta = np.asarray(beta, dtype=np.float32)
    Wo = np.asarray(Wo, dtype=np.float32)
    bo = np.asarray(bo, dtype=np.float32)

    # ---- host routing replica (control plane: packing only) ----
    logits = z @ Wr + br
    m = logits.max(-1, keepdims=True)
    ex = np.exp((logits - m) / TAU)
    probs_h = ex / ex.sum(-1, keepdims=True)
    order = np.argsort(-probs_h, axis=-1, kind="stable")
    top2 = order[:, :2]

    counts = np.zeros((NCORES, E), dtype=np.int64)
    for c in range(NCORES):
        t2 = top2[c * NTOK : (c + 1) * NTOK]
        for e in range(E):
            counts[c, e] = (t2 == e).sum()
    C = int(((counts.max() + 127) // 128) * 128)

    # packed layouts + slot maps
    zT = np.ascontiguousarray(z.T)  # [D, N]
    gwo = (gamma * Wo[:, :, 0])  # [E, H]
    statlhs = np.stack([np.ones((E, D), np.float32), gwo], axis=-1).astype(np.float32)
    swo = gwo.sum(-1).astype(np.float32)
    bop = (bo[:, 0] + (beta * Wo[:, :, 0]).sum(-1)).astype(np.float32)

    # pre-tiled weight layouts (match SBUF tile layout -> contiguous DMAs)
    RG, NG = 2, TT // 2
    w1_t = np.ascontiguousarray(W1.reshape(E, KC, P, H).transpose(0, 2, 1, 3))
    w2_t = np.ascontiguousarray(W2.reshape(E, KC, P, H).transpose(0, 2, 1, 3))
    sl_t = np.ascontiguousarray(statlhs.reshape(E, KC, P, 2).transpose(0, 2, 1, 3))
    b1_t = np.ascontiguousarray(b1.reshape(E, H // P, P).transpose(0, 2, 1))
    b2_t = np.ascontiguousarray(b2.reshape(E, H // P, P).transpose(0, 2, 1))

    def _pack_blocks(zTp):
        Zp = zTp.reshape(KC, P, E * C)
        blocks = []
        for e in range(E):
            for (c0, tw) in _chunks(C):
                blocks.append(
                    Zp[:, :, e * C + c0 : e * C + c0 + tw].transpose(1, 0, 2).reshape(-1)
                )
        return np.concatenate(blocks)

    in_maps = []
    slot_of = np.zeros((N, 2), dtype=np.int64)  # global slot per (token, k)
    for c in range(NCORES):
        lo = c * NTOK
        t2 = top2[lo : lo + NTOK]
        zTc = zT[:, lo : lo + NTOK]
        zrt = np.ascontiguousarray(
            zTc.reshape(KC, P, NG, RG * P).transpose(2, 1, 0, 3)
        )
        zTp = np.zeros((D, E * C), dtype=np.float32)
        for e in range(E):
            rows, ks = np.nonzero(t2 == e)
            zTp[:, e * C : e * C + rows.size] = zTc[:, rows]
            slot_of[lo + rows, ks] = c * (E * C) + e * C + np.arange(rows.size)
        in_maps.append({
            "zrt": zrt, "zTp": _pack_blocks(zTp), "w1": w1_t, "w2": w2_t,
            "wr": Wr, "br": br, "b1": b1_t, "b2": b2_t, "statlhs": sl_t,
            "swo": swo, "bop": bop,
        })

    key = ("nc", C)
    if key not in _cache:
        _cache[key] = build_kernel(C)
    nc = _cache[key]
    global _last_in_maps
    _last_in_maps = in_maps

    res = run_bass_kernel_spmd(nc, in_maps, core_ids=list(range(NCORES)))

    probs = np.concatenate([r["probs"] for r in res.results], axis=0)
    topk_idx = np.concatenate([r["topk"] for r in res.results], axis=0).astype(np.int32)
    gates = np.concatenate([r["gates"] for r in res.results], axis=0)
    ye = np.concatenate([r["ye"].reshape(-1) for r in res.results], axis=0)  # [NCORES*E*C]
    il = np.stack([r["implood"][0].reshape(TT, 2 * E).sum(0) for r in res.results], axis=0)

    y_hat = (gates[:, 0] * ye[slot_of[:, 0]] + gates[:, 1] * ye[slot_of[:, 1]])
    y_hat = y_hat.astype(np.float32)[:, None]
    importance = (il[:, :E].sum(0) / N).astype(np.float32)
    load = (il[:, E:].sum(0) / N).astype(np.float32)
    return y_hat, probs, topk_idx, importance, load
